# revision 7
# baseline (speedup 1.0000x reference)
"""CrossAttention Trainium2 kernel (v4).

Full inputs in, full output out. Data-parallel over batch: core b computes
batch item b of 8.

Per-core math (layouts transposed so the PE contraction dim is always the
partition dim, no on-chip transposes):
  V[k, d]   = (kv_b @ Wv^T)            8 k-chunks, augmented with a ones col
  QT[d, q]  = (Wq*scale @ q_b^T)       per head-pair p (d = pair dims)
  KT[d, k]  = (Wk @ kv_b^T)
  S^T[k, q] = K Q^T                    per head; the two heads of a pair run
                                       row-tiled (rows 0-63 / 64-127)
  P^T = exp(S^T - ln256) * epos^T      epos precomputed on host, bf16
  O^T[d, q] (+rowsum row 64) = V_aug^T P^T, psum-accumulated over k
  X^T = O^T[0:64] * (1/rowsum)
  out[q, e] = X^T.T @ WprojT + bias

Scheduling: input DMAs are ordered so the pair-0 attention chain starts as
early as possible (kv, first halves of wv/wq/wk, q first; second halves
stream in under attention). V heads 8-11 and the QK projections for pair
p+1 are emitted inside pair p's k-loop as PE gap filler, which also keeps
the HAM clock gate at 8/8. A warmup matmul burst covers the initial DMA
wait. All matmul operands are 16-bit; PSUM accumulation is fp32.
"""

import numpy as np

B, L, DIM, H, HD = 8, 1024, 768, 12, 64
NCORES = 8
CP = DIM // 128  # 6 chunks of the contraction/feature dim
KC = L // 128    # 8 k-chunks
NP = H // 2      # 6 head pairs
SCALE = HD ** -0.5
LN_OFF = float(np.log(256.0))

_CACHE = {}


def _build():
    import concourse.bass as bass
    import concourse.mybir as mybir
    import concourse.tile as tile
    from concourse import bacc

    f32 = mybir.dt.float32
    f16 = mybir.dt.float16
    bf16 = mybir.dt.bfloat16
    AF = mybir.ActivationFunctionType

    nc = bacc.Bacc("TRN2", target_bir_lowering=False, debug=False)

    qT = nc.dram_tensor("qT", [DIM, L], f16, kind="ExternalInput")
    kvT = nc.dram_tensor("kvT", [DIM, L], f16, kind="ExternalInput")
    wq = nc.dram_tensor("wq", [DIM, DIM], f16, kind="ExternalInput")  # [c, d]
    wk = nc.dram_tensor("wk", [DIM, DIM], f16, kind="ExternalInput")  # [c, d]
    wv = nc.dram_tensor("wv", [DIM, DIM], f16, kind="ExternalInput")  # [c, d]
    wp = nc.dram_tensor("wp", [DIM, DIM], f16, kind="ExternalInput")  # [d, e]
    bias = nc.dram_tensor("bias", [128, DIM], f32, kind="ExternalInput")
    epos = nc.dram_tensor("epos", [H, L, L], bf16, kind="ExternalInput")  # [h,k,q]
    out = nc.dram_tensor("out", [L, DIM], f16, kind="ExternalOutput")
    rscr = nc.dram_tensor("rs_scratch", [H, L], f32)

    with tile.TileContext(nc) as tc:
        with tc.tile_pool(name="persist", bufs=1) as persist:
            QT = persist.tile([128, NP, L], f16)   # pair p: heads 2p, 2p+1
            KT = persist.tile([128, NP, L], f16)
            XT = persist.tile([128, NP, L], f16)
            Vt = [
                persist.tile([128, H, HD + 1], f16, name=f"Vt{k}", tag=f"V{k}")
                for k in range(KC)
            ]
            wp_sb = persist.tile([128, CP, DIM], f16)
            bias_bc = persist.tile([128, DIM], f32)
            rs_a = persist.tile([6, L], f32)
            rs_b = persist.tile([4, L], f32)
            rs_c = persist.tile([2, L], f32)
            recip_a = persist.tile([6, L], f32)
            recip_b = persist.tile([4, L], f32)
            recip_c = persist.tile([2, L], f32)
            expb = persist.tile([128, 1], f32)
            nc.vector.memset(expb[:], -LN_OFF)
            warm_w = persist.tile([128, 128], f16)
            warm_x = persist.tile([128, 512], f16)
            nc.vector.memset(warm_w[:], 0.0)
            nc.vector.memset(warm_x[:], 0.0)

            q_sb = persist.tile([128, CP, L], f16)
            kv_sb = persist.tile([128, CP, L], f16)
            wq_sb = persist.tile([128, CP, DIM], f16)
            wk_sb = persist.tile([128, CP, DIM], f16)
            wv_sb = persist.tile([128, CP, DIM], f16)

            kv_r = kvT.rearrange("(a p) q -> p a q", p=128)
            q_r = qT.rearrange("(a p) q -> p a q", p=128)
            wq_r = wq.rearrange("(a p) d -> p a d", p=128)
            wk_r = wk.rearrange("(a p) d -> p a d", p=128)
            wv_r = wv.rearrange("(a p) d -> p a d", p=128)
            wp_r = wp.rearrange("(a p) d -> p a d", p=128)
            # DMA order: everything pair-0 attention needs first (kv, first
            # halves of wv/wq/wk covering heads 0-7, q), then the second
            # halves + wp/bias stream in under attention.
            for c in range(CP):
                nc.sync.dma_start(kv_sb[:, c, :], kv_r[:, c, :])
            for c in range(CP):
                nc.sync.dma_start(wv_sb[:, c, 0:512], wv_r[:, c, 0:512])
            for c in range(CP):
                nc.sync.dma_start(q_sb[:, c, :], q_r[:, c, :])
            for c in range(CP):
                nc.sync.dma_start(wq_sb[:, c, 0:512], wq_r[:, c, 0:512])
                nc.sync.dma_start(wk_sb[:, c, 0:512], wk_r[:, c, 0:512])
            for c in range(CP):
                nc.sync.dma_start(wq_sb[:, c, 512:768], wq_r[:, c, 512:768])
                nc.sync.dma_start(wk_sb[:, c, 512:768], wk_r[:, c, 512:768])
                nc.sync.dma_start(wv_sb[:, c, 512:768], wv_r[:, c, 512:768])
            for c in range(CP):
                nc.sync.dma_start(wp_sb[:, c, :], wp_r[:, c, :])
            nc.sync.dma_start(bias_bc[:], bias[:])

            with (
                tc.tile_pool(name="psA", bufs=2, space="PSUM") as psA,
                tc.tile_pool(name="psO", bufs=2, space="PSUM") as psO,
                tc.tile_pool(name="eposp", bufs=7) as eposp,
                tc.tile_pool(name="praw", bufs=3) as praw,
                tc.tile_pool(name="ptp", bufs=3) as ptp,
                tc.tile_pool(name="xtup", bufs=8) as xtup,
                tc.tile_pool(name="bcp", bufs=3) as bcp,
            ):
                # warmup burst: serialized matmuls on zeros during the input
                # DMA wall so the PE HAM gate is at 8/8 when real work lands
                wps = psA.tile([128, 512], f32, tag="psA")
                for _ in range(30):
                    nc.tensor.matmul(wps[:], warm_w[:], warm_x[:])

                # V projection, heads 0-7 (d 0-511); heads 8-11 are deferred
                # into the pair loop (not needed until pair 4)
                for k in range(KC):
                    ps = psA.tile([128, 512], f32, tag="psA", name=f"va{k}")
                    for c in range(CP):
                        nc.tensor.matmul(
                            ps[:],
                            kv_sb[:, c, k * 128:(k + 1) * 128],
                            wv_sb[:, c, 0:512],
                            start=(c == 0),
                            stop=(c == CP - 1),
                        )
                    nc.vector.memset(Vt[k][:, :, HD:HD + 1], 1.0)
                    nc.vector.tensor_copy(
                        Vt[k][:, 0:8, 0:HD],
                        ps[:].rearrange("p (h d) -> p h d", d=HD),
                    )

                # QK projection for pair 0
                for w_sb, x_sb, dst in ((wq_sb, q_sb, QT), (wk_sb, kv_sb, KT)):
                    ps = psA.tile([128, L], f32, tag="psA")
                    for c in range(CP):
                        for hf in range(2):
                            nc.tensor.matmul(
                                ps[:, hf * 512:(hf + 1) * 512],
                                w_sb[:, c, 0:128],
                                x_sb[:, c, hf * 512:(hf + 1) * 512],
                                start=(c == 0),
                                stop=(c == CP - 1),
                            )
                    nc.vector.tensor_copy(dst[:, 0, :], ps[:])

                xtu = [None] * H

                def normalize(h):
                    p, sub = divmod(h, 2)
                    bc = bcp.tile([64, L], f32, name=f"bc{h}", tag="bc")
                    nc.sync.dma_start(bc[:], rscr[h:h + 1, :].broadcast_to([64, L]))
                    nc.vector.tensor_mul(
                        XT[sub * 64:(sub + 1) * 64, p, :],
                        xtu[h][0:64, :],
                        bc[:],
                    )

                # deferred V second half: per k, one [128,256] psum fill for
                # heads 8-11; queued as PE filler inside pairs 0-1
                vb_state = {}

                def vb_fill(k):
                    ps = psA.tile([128, 256], f32, tag="psA", name=f"vb{k}")
                    vb_state[k] = ps
                    for c in range(CP):
                        nc.tensor.matmul(
                            ps[:],
                            kv_sb[:, c, k * 128:(k + 1) * 128],
                            wv_sb[:, c, 512:768],
                            start=(c == 0),
                            stop=(c == CP - 1),
                        )

                def vb_evict(k):
                    nc.vector.tensor_copy(
                        Vt[k][:, 8:12, 0:HD],
                        vb_state[k][:].rearrange("p (h d) -> p h d", d=HD),
                    )

                def proj_burst(p_dst, w_sb, x_sb, dst):
                    """12 dense matmuls projecting pair p_dst, then evict."""
                    ps_t = psA.tile([128, L], f32, tag="psA", name=f"pj{p_dst}")
                    for c in range(CP):
                        for hf in range(2):
                            nc.tensor.matmul(
                                ps_t[:, hf * 512:(hf + 1) * 512],
                                w_sb[:, c, p_dst * 128:(p_dst + 1) * 128],
                                x_sb[:, c, hf * 512:(hf + 1) * 512],
                                start=(c == 0),
                                stop=(c == CP - 1),
                            )
                    nc.vector.tensor_copy(dst[:, p_dst, :], ps_t[:])

                # ---------------- attention, pair at a time ----------------
                for p in range(NP):
                    h0, h1 = 2 * p, 2 * p + 1
                    o_ps0 = psO.tile([HD + 1, L], f32, tag="psO")
                    o_ps1 = psO.tile([HD + 1, L], f32, tag="psO")
                    # short dense bursts of independent PE work per k slot
                    bursts = {}
                    if p + 1 < NP:
                        bursts[1] = lambda: proj_burst(p + 1, wq_sb, q_sb, QT)
                        bursts[4] = lambda: proj_burst(p + 1, wk_sb, kv_sb, KT)
                    if p < 2:
                        for i, vk in enumerate(range(4) if p == 0 else range(4, KC)):
                            bursts[(2, 3, 5, 6)[i]] = (
                                lambda vk=vk: (vb_fill(vk), vb_evict(vk))
                            )

                    for k in range(KC):
                        s0 = psA.tile([128, L], f32, tag="psA")
                        s1 = psA.tile([128, L], f32, tag="psA")
                        kt_sl = slice(k * 128, (k + 1) * 128)
                        for hf in range(2):
                            qs = slice(hf * 512, (hf + 1) * 512)
                            nc.tensor.matmul(
                                s0[:, qs], KT[0:64, p, kt_sl], QT[0:64, p, qs],
                            )
                            nc.tensor.matmul(
                                s1[:, qs], KT[64:128, p, kt_sl], QT[64:128, p, qs],
                            )
                        for h, s_ps in ((h0, s0), (h1, s1)):
                            pr = praw.tile([128, L], bf16, tag="pr")
                            nc.scalar.activation(pr[:], s_ps[:], AF.Exp, bias=expb[:])
                            ep = eposp.tile([128, L], bf16, tag="ep")
                            nc.sync.dma_start(ep[:], epos[h, kt_sl, :])
                            pt = ptp.tile([128, L], bf16, tag="pt")
                            nc.vector.tensor_mul(pt[:], pr[:], ep[:])
                            o_ps = o_ps0 if h == h0 else o_ps1
                            for hf in range(2):
                                qs = slice(hf * 512, (hf + 1) * 512)
                                nc.tensor.matmul(
                                    o_ps[:, qs],
                                    Vt[k][:, h, :],
                                    pt[:, qs],
                                    start=(k == 0),
                                    stop=(k == KC - 1),
                                )
                        if k in bursts:
                            bursts[k]()
                    for h, o_ps in ((h0, o_ps0), (h1, o_ps1)):
                        xtu[h] = xtup.tile([HD + 1, L], f32, name=f"xtu{h}", tag="xtu")
                        nc.vector.tensor_copy(xtu[h][:, 0:512], o_ps[:, 0:512])
                        nc.vector.tensor_copy(xtu[h][:, 512:L], o_ps[:, 512:L])
                        if h < 6:
                            rs_t, row = rs_a, h
                        elif h < 10:
                            rs_t, row = rs_b, h - 6
                        else:
                            rs_t, row = rs_c, h - 10
                        nc.sync.dma_start(
                            rs_t[row:row + 1, :], xtu[h][HD:HD + 1, :]
                        )
                    if p == 2:
                        nc.vector.reciprocal_approx_fast(recip_a[:], rs_a[:])
                        nc.sync.dma_start(rscr[0:6, :], recip_a[:])
                        for hh in range(6):
                            normalize(hh)
                    if p == 4:
                        nc.vector.reciprocal_approx_fast(recip_b[:], rs_b[:])
                        nc.sync.dma_start(rscr[6:10, :], recip_b[:])
                        for hh in range(6, 10):
                            normalize(hh)
                    if p == NP - 1:
                        nc.vector.reciprocal_approx_fast(recip_c[:], rs_c[:])
                        nc.sync.dma_start(rscr[10:12, :], recip_c[:])
                        for hh in range(10, 12):
                            normalize(hh)

            # ---------------- output projection ----------------
            with (
                tc.tile_pool(name="outp", bufs=2) as outp,
                tc.tile_pool(name="psOut", bufs=2, space="PSUM") as psOut,
            ):
                for qc in range(KC):
                    ps = psOut.tile([128, DIM], f32)
                    for d in range(CP):
                        for lo, sz in ((0, 512), (512, 256)):
                            nc.tensor.matmul(
                                ps[:, lo:lo + sz],
                                XT[:, d, qc * 128:(qc + 1) * 128],
                                wp_sb[:, d, lo:lo + sz],
                                start=(d == 0),
                                stop=(d == CP - 1),
                            )
                    ot = outp.tile([128, DIM], f16)
                    nc.vector.tensor_add(ot[:], ps[:], bias_bc[:])
                    nc.sync.dma_start(out[qc * 128:(qc + 1) * 128, :], ot[:])

    nc.compile()
    return nc


def _get_nc():
    if "nc" not in _CACHE:
        _CACHE["nc"] = _build()
    return _CACHE["nc"]


def _host_prep(q, kv, attn_pos, Wq, Wkv, Wproj, bproj):
    import ml_dtypes

    q = np.asarray(q, dtype=np.float32)
    kv = np.asarray(kv, dtype=np.float32)
    attn_pos = np.asarray(attn_pos, dtype=np.float32)
    Wq = np.asarray(Wq, dtype=np.float32)
    Wkv = np.asarray(Wkv, dtype=np.float32)
    Wproj = np.asarray(Wproj, dtype=np.float32)
    bproj = np.asarray(bproj, dtype=np.float32)

    wq16 = np.ascontiguousarray((Wq * SCALE).T).astype(np.float16)   # [c, d]
    wk16 = np.ascontiguousarray(Wkv[:DIM].T).astype(np.float16)      # [c, d]
    wv16 = np.ascontiguousarray(Wkv[DIM:].T).astype(np.float16)      # [c, d]
    wp16 = np.ascontiguousarray(Wproj.T).astype(np.float16)          # [d, e]
    bias = np.ascontiguousarray(np.tile(bproj[None, :], (128, 1)))
    # epos[h, k, q] = exp(attn_pos[0, h, q, k])
    epos = np.ascontiguousarray(
        np.exp(attn_pos[0]).transpose(0, 2, 1)
    ).astype(ml_dtypes.bfloat16)

    qT = np.ascontiguousarray(q.transpose(0, 2, 1)).astype(np.float16)
    kvT = np.ascontiguousarray(kv.transpose(0, 2, 1)).astype(np.float16)

    shared = {
        "wq": wq16, "wk": wk16, "wv": wv16, "wp": wp16,
        "bias": bias, "epos": epos,
    }
    in_maps = []
    for b in range(B):
        m = dict(shared)
        m["qT"] = qT[b]
        m["kvT"] = kvT[b]
        in_maps.append(m)
    return in_maps


def kernel(q, kv, attn_pos, Wq, Wkv, Wproj, bproj):
    from concourse.bass_utils import run_bass_kernel_spmd

    nc = _get_nc()
    in_maps = _host_prep(q, kv, attn_pos, Wq, Wkv, Wproj, bproj)
    res = run_bass_kernel_spmd(nc, in_maps, list(range(NCORES)))
    return np.stack(
        [res.results[b]["out"].astype(np.float32) for b in range(B)], axis=0
    )


# revision 8
# speedup vs baseline: 1.1131x; 1.1131x over previous
"""CrossAttention Trainium2 kernel (v5).

Full inputs in, full output out. Data-parallel over batch: core b computes
batch item b of 8.

Per-core math (layouts transposed so the PE contraction dim is always the
partition dim, no on-chip transposes):
  V[k, d]   = (kv_b @ Wv^T)            8 k-chunks, augmented with a ones col
  QT[d, q]  = (Wq*scale @ q_b^T)       per head-pair p (d = pair dims)
  KT[d, k]  = (Wk @ kv_b^T)
  S^T[k, q] = K Q^T                    per head; the two heads of a pair run
                                       row-tiled (rows 0-63 / 64-127)
  P^T = exp(S^T - ln256) * epos^T      epos precomputed on host, bf16
  O^T[d, q] (+rowsum row 64) = V_aug^T P^T, psum-accumulated over k
  X^T = O^T[0:64] * (1/rowsum)
  out[q, e] = X^T.T @ WprojT + bias

Scheduling: whole-tensor input DMAs in consumption order; a warmup matmul
burst keeps the PE HAM clock gate at 8/8 through the initial DMA wait; the
QK projections for pair p+1 are spread through pair p's k-loop as PE gap
filler; rowsum reciprocals run in 3 batches so the final one is tiny; all
matmul operands are 16-bit with fp32 PSUM accumulation.
"""

import numpy as np

B, L, DIM, H, HD = 8, 1024, 768, 12, 64
NCORES = 8
CP = DIM // 128  # 6 chunks of the contraction/feature dim
KC = L // 128    # 8 k-chunks
NP = H // 2      # 6 head pairs
SCALE = HD ** -0.5
LN_OFF = float(np.log(256.0))

_CACHE = {}


def _build():
    import concourse.bass as bass
    import concourse.mybir as mybir
    import concourse.tile as tile
    from concourse import bacc

    f32 = mybir.dt.float32
    f16 = mybir.dt.float16
    bf16 = mybir.dt.bfloat16
    AF = mybir.ActivationFunctionType

    nc = bacc.Bacc("TRN2", target_bir_lowering=False, debug=False)

    qT = nc.dram_tensor("qT", [DIM, L], f16, kind="ExternalInput")
    kvT = nc.dram_tensor("kvT", [DIM, L], f16, kind="ExternalInput")
    wq = nc.dram_tensor("wq", [DIM, DIM], f16, kind="ExternalInput")  # [c, d]
    wk = nc.dram_tensor("wk", [DIM, DIM], f16, kind="ExternalInput")  # [c, d]
    wv = nc.dram_tensor("wv", [DIM, DIM], f16, kind="ExternalInput")  # [c, d]
    wp = nc.dram_tensor("wp", [DIM, DIM], f16, kind="ExternalInput")  # [d, e]
    bias = nc.dram_tensor("bias", [128, DIM], f32, kind="ExternalInput")
    epos = nc.dram_tensor("epos", [H, L, L], bf16, kind="ExternalInput")  # [h,k,q]
    out = nc.dram_tensor("out", [L, DIM], f16, kind="ExternalOutput")
    rscr = nc.dram_tensor("rs_scratch", [H, L], f32)

    with tile.TileContext(nc) as tc:
        with tc.tile_pool(name="persist", bufs=1) as persist:
            QT = persist.tile([128, NP, L], f16)   # pair p: heads 2p, 2p+1
            KT = persist.tile([128, NP, L], f16)
            XT = persist.tile([128, NP, L], f16)
            Vt = [
                persist.tile([128, H, HD + 1], f16, name=f"Vt{k}", tag=f"V{k}")
                for k in range(KC)
            ]
            wp_sb = persist.tile([128, CP, DIM], f16)
            bias_bc = persist.tile([128, DIM], f32)
            rs_a = persist.tile([6, L], f32)
            rs_b = persist.tile([4, L], f32)
            rs_c = persist.tile([2, L], f32)
            recip_a = persist.tile([6, L], f32)
            recip_b = persist.tile([4, L], f32)
            recip_c = persist.tile([2, L], f32)
            expb = persist.tile([128, 1], f32)
            nc.vector.memset(expb[:], -LN_OFF)
            warm_w = persist.tile([128, 128], f16)
            warm_x = persist.tile([128, 512], f16)
            nc.vector.memset(warm_w[:], 0.0)
            nc.vector.memset(warm_x[:], 0.0)

            q_sb = persist.tile([128, CP, L], f16)
            kv_sb = persist.tile([128, CP, L], f16)
            wq_sb = persist.tile([128, CP, DIM], f16)
            wk_sb = persist.tile([128, CP, DIM], f16)
            wv_sb = persist.tile([128, CP, DIM], f16)

            # whole-tensor input DMAs (large transfers run ~341 GB/s vs
            # ~250 for 256 KB chunks), ordered by first consumption
            nc.sync.dma_start(kv_sb[:], kvT.rearrange("(a p) q -> p a q", p=128))
            nc.sync.dma_start(wv_sb[:], wv.rearrange("(a p) d -> p a d", p=128))
            nc.sync.dma_start(q_sb[:], qT.rearrange("(a p) q -> p a q", p=128))
            nc.sync.dma_start(wq_sb[:], wq.rearrange("(a p) d -> p a d", p=128))
            nc.sync.dma_start(wk_sb[:], wk.rearrange("(a p) d -> p a d", p=128))
            nc.sync.dma_start(wp_sb[:], wp.rearrange("(a p) d -> p a d", p=128))
            nc.sync.dma_start(bias_bc[:], bias[:])

            with (
                tc.tile_pool(name="psA", bufs=2, space="PSUM") as psA,
                tc.tile_pool(name="psO", bufs=2, space="PSUM") as psO,
                tc.tile_pool(name="eposp", bufs=7) as eposp,
                tc.tile_pool(name="praw", bufs=3) as praw,
                tc.tile_pool(name="ptp", bufs=3) as ptp,
                tc.tile_pool(name="xtup", bufs=8) as xtup,
                tc.tile_pool(name="bcp", bufs=3) as bcp,
            ):
                # warmup burst: serialized matmuls on zeros during the input
                # DMA wall so the PE HAM gate is at 8/8 when real work lands
                wps = psA.tile([128, 512], f32, tag="psA")
                for _ in range(30):
                    nc.tensor.matmul(wps[:], warm_w[:], warm_x[:])

                # ---------------- V projection ----------------
                for k in range(KC):
                    ps = psA.tile([128, L], f32, tag="psA")
                    for c in range(CP):
                        for lo, sz in ((0, 512), (512, 256)):
                            nc.tensor.matmul(
                                ps[:, lo:lo + sz],
                                kv_sb[:, c, k * 128:(k + 1) * 128],
                                wv_sb[:, c, lo:lo + sz],
                                start=(c == 0),
                                stop=(c == CP - 1),
                            )
                    nc.vector.memset(Vt[k][:, :, HD:HD + 1], 1.0)
                    nc.vector.tensor_copy(
                        Vt[k][:, :, 0:HD],
                        ps[:, 0:DIM].rearrange("p (h d) -> p h d", d=HD),
                    )

                # QK projection for pair 0
                for w_sb, x_sb, dst in ((wq_sb, q_sb, QT), (wk_sb, kv_sb, KT)):
                    ps = psA.tile([128, L], f32, tag="psA")
                    for c in range(CP):
                        for hf in range(2):
                            nc.tensor.matmul(
                                ps[:, hf * 512:(hf + 1) * 512],
                                w_sb[:, c, 0:128],
                                x_sb[:, c, hf * 512:(hf + 1) * 512],
                                start=(c == 0),
                                stop=(c == CP - 1),
                            )
                    nc.vector.tensor_copy(dst[:, 0, :], ps[:])

                xtu = [None] * H

                def normalize(h):
                    p, sub = divmod(h, 2)
                    bc = bcp.tile([64, L], f32, name=f"bc{h}", tag="bc")
                    nc.sync.dma_start(bc[:], rscr[h:h + 1, :].broadcast_to([64, L]))
                    nc.vector.tensor_mul(
                        XT[sub * 64:(sub + 1) * 64, p, :],
                        xtu[h][0:64, :],
                        bc[:],
                    )

                # ---------------- attention, pair at a time ----------------
                for p in range(NP):
                    h0, h1 = 2 * p, 2 * p + 1
                    o_ps0 = psO.tile([HD + 1, L], f32, tag="psO")
                    o_ps1 = psO.tile([HD + 1, L], f32, tag="psO")
                    proj_jobs = []
                    if p + 1 < NP:
                        qps = psA.tile([128, L], f32, tag="psA", name=f"qp{p}")
                        kps = psA.tile([128, L], f32, tag="psA", name=f"kp{p}")
                        for ps_t, w_sb, x_sb in (
                            (qps, wq_sb, q_sb), (kps, wk_sb, kv_sb),
                        ):
                            for c in range(CP):
                                for hf in range(2):
                                    proj_jobs.append((ps_t, w_sb, x_sb, c, hf))
                    nj = 0

                    def drain_proj(n):
                        nonlocal nj
                        for _ in range(n):
                            if nj >= len(proj_jobs):
                                return
                            ps_t, w_sb, x_sb, c, hf = proj_jobs[nj]
                            nc.tensor.matmul(
                                ps_t[:, hf * 512:(hf + 1) * 512],
                                w_sb[:, c, (p + 1) * 128:(p + 2) * 128],
                                x_sb[:, c, hf * 512:(hf + 1) * 512],
                                start=(c == 0),
                                stop=(c == CP - 1),
                            )
                            nj += 1

                    for k in range(KC):
                        s0 = psA.tile([128, L], f32, tag="psA")
                        s1 = psA.tile([128, L], f32, tag="psA")
                        kt_sl = slice(k * 128, (k + 1) * 128)
                        for hf in range(2):
                            qs = slice(hf * 512, (hf + 1) * 512)
                            nc.tensor.matmul(
                                s0[:, qs], KT[0:64, p, kt_sl], QT[0:64, p, qs],
                            )
                            nc.tensor.matmul(
                                s1[:, qs], KT[64:128, p, kt_sl], QT[64:128, p, qs],
                            )
                        drain_proj(2)
                        for h, s_ps in ((h0, s0), (h1, s1)):
                            pr = praw.tile([128, L], bf16, tag="pr")
                            nc.scalar.activation(pr[:], s_ps[:], AF.Exp, bias=expb[:])
                            ep = eposp.tile([128, L], bf16, tag="ep")
                            nc.sync.dma_start(ep[:], epos[h, kt_sl, :])
                            pt = ptp.tile([128, L], bf16, tag="pt")
                            nc.vector.tensor_mul(pt[:], pr[:], ep[:])
                            o_ps = o_ps0 if h == h0 else o_ps1
                            for hf in range(2):
                                qs = slice(hf * 512, (hf + 1) * 512)
                                nc.tensor.matmul(
                                    o_ps[:, qs],
                                    Vt[k][:, h, :],
                                    pt[:, qs],
                                    start=(k == 0),
                                    stop=(k == KC - 1),
                                )
                            drain_proj(1)
                        if k == 3 and proj_jobs:
                            nc.vector.tensor_copy(QT[:, p + 1, :], qps[:])
                        if k == 6 and proj_jobs:
                            nc.vector.tensor_copy(KT[:, p + 1, :], kps[:])
                    for h, o_ps in ((h0, o_ps0), (h1, o_ps1)):
                        xtu[h] = xtup.tile([HD + 1, L], f32, name=f"xtu{h}", tag="xtu")
                        nc.vector.tensor_copy(xtu[h][:, 0:512], o_ps[:, 0:512])
                        nc.vector.tensor_copy(xtu[h][:, 512:L], o_ps[:, 512:L])
                        if h < 6:
                            rs_t, row = rs_a, h
                        elif h < 10:
                            rs_t, row = rs_b, h - 6
                        else:
                            rs_t, row = rs_c, h - 10
                        nc.sync.dma_start(
                            rs_t[row:row + 1, :], xtu[h][HD:HD + 1, :]
                        )
                    if p == 2:
                        nc.vector.reciprocal_approx_fast(recip_a[:], rs_a[:])
                        nc.sync.dma_start(rscr[0:6, :], recip_a[:])
                        for hh in range(6):
                            normalize(hh)
                    if p == 4:
                        nc.vector.reciprocal_approx_fast(recip_b[:], rs_b[:])
                        nc.sync.dma_start(rscr[6:10, :], recip_b[:])
                        for hh in range(6, 10):
                            normalize(hh)
                    if p == NP - 1:
                        nc.vector.reciprocal_approx_fast(recip_c[:], rs_c[:])
                        nc.sync.dma_start(rscr[10:12, :], recip_c[:])
                        for hh in range(10, 12):
                            normalize(hh)

            # ---------------- output projection ----------------
            with (
                tc.tile_pool(name="outp", bufs=2) as outp,
                tc.tile_pool(name="psOut", bufs=2, space="PSUM") as psOut,
            ):
                for qc in range(KC):
                    ps = psOut.tile([128, DIM], f32)
                    for d in range(CP):
                        for lo, sz in ((0, 512), (512, 256)):
                            nc.tensor.matmul(
                                ps[:, lo:lo + sz],
                                XT[:, d, qc * 128:(qc + 1) * 128],
                                wp_sb[:, d, lo:lo + sz],
                                start=(d == 0),
                                stop=(d == CP - 1),
                            )
                    ot = outp.tile([128, DIM], f16)
                    nc.vector.tensor_add(ot[:], ps[:], bias_bc[:])
                    nc.sync.dma_start(out[qc * 128:(qc + 1) * 128, :], ot[:])

    nc.compile()
    return nc


def _get_nc():
    if "nc" not in _CACHE:
        _CACHE["nc"] = _build()
    return _CACHE["nc"]


def _host_prep(q, kv, attn_pos, Wq, Wkv, Wproj, bproj):
    import ml_dtypes

    q = np.asarray(q, dtype=np.float32)
    kv = np.asarray(kv, dtype=np.float32)
    attn_pos = np.asarray(attn_pos, dtype=np.float32)
    Wq = np.asarray(Wq, dtype=np.float32)
    Wkv = np.asarray(Wkv, dtype=np.float32)
    Wproj = np.asarray(Wproj, dtype=np.float32)
    bproj = np.asarray(bproj, dtype=np.float32)

    wq16 = np.ascontiguousarray((Wq * SCALE).T).astype(np.float16)   # [c, d]
    wk16 = np.ascontiguousarray(Wkv[:DIM].T).astype(np.float16)      # [c, d]
    wv16 = np.ascontiguousarray(Wkv[DIM:].T).astype(np.float16)      # [c, d]
    wp16 = np.ascontiguousarray(Wproj.T).astype(np.float16)          # [d, e]
    bias = np.ascontiguousarray(np.tile(bproj[None, :], (128, 1)))
    # epos[h, k, q] = exp(attn_pos[0, h, q, k])
    epos = np.ascontiguousarray(
        np.exp(attn_pos[0]).transpose(0, 2, 1)
    ).astype(ml_dtypes.bfloat16)

    qT = np.ascontiguousarray(q.transpose(0, 2, 1)).astype(np.float16)
    kvT = np.ascontiguousarray(kv.transpose(0, 2, 1)).astype(np.float16)

    shared = {
        "wq": wq16, "wk": wk16, "wv": wv16, "wp": wp16,
        "bias": bias, "epos": epos,
    }
    in_maps = []
    for b in range(B):
        m = dict(shared)
        m["qT"] = qT[b]
        m["kvT"] = kvT[b]
        in_maps.append(m)
    return in_maps


def kernel(q, kv, attn_pos, Wq, Wkv, Wproj, bproj):
    from concourse.bass_utils import run_bass_kernel_spmd

    nc = _get_nc()
    in_maps = _host_prep(q, kv, attn_pos, Wq, Wkv, Wproj, bproj)
    res = run_bass_kernel_spmd(nc, in_maps, list(range(NCORES)))
    return np.stack(
        [res.results[b]["out"].astype(np.float32) for b in range(B)], axis=0
    )
